# revision 38
# baseline (speedup 1.0000x reference)
"""Trainium2 Bass kernel for the Brill-Lindquist Christoffel-symbol grid.

Math: the reference reduces to
    psi  = 1 + sum_n m_n / (2 r_n),   m = softplus(pre)
    h    = psi^4
    G_c  = finite-difference gradient of h along grid axis c (2nd order
           central interior, 1st order one-sided edges, spacing DX)
    W_c  = 0.5 * G_c / h
    Gamma^i_{jk} = delta_ij W_k + delta_ik W_j - delta_jk W_i
so the [96,96,96,3,3,3] output is +-W_c scattered over 27 slots per
point (21 nonzero, 6 identically zero).

Sharding: axis 0 (12 planes per core x 8 cores). h is analytic in the
inputs, so each core evaluates its slab plus a 1-plane halo directly --
no inter-core exchange. Per core the grid is row-packed: row = a0*96+a1
(1152 rows -> 9 tiles of 128 partitions), free dim = a2 (96); h lives on
an 11-tile extended row window (halo tiles at both ends).

r^2 = (x-px)^2+(y-py)^2 + (z-pz)^2 is an outer sum of a per-row and a
per-z term, so it is produced by tiny 2-row matmuls on the otherwise
idle tensor engine (lhsT = [ab_n | 1], rhs = [1 | crow_n]). The h field
is built once per core in fp32 (axis-2 shift FD, 1/h) and bf16 (matmul
operand). Axis-0/1 derivatives are bf16 matmuls against per-core FD
matrices (+-0.25/DX, +-0.5/DX exact in bf16; one-sided grid edges
folded in). bf16 rounding of h bounds the W error by ~2^-10/DX ~ 0.05
absolute vs the ~500 the 2e-2 gate allows.

Output: device stores only the 21 nonzero slots, slot-major bf16
([row, s*96+z], 4032 B/row); the host inserts the 6 zero slots, casts
to f32 and permutes to [...,z,3,3,3]. The compressed slot order is
value-major -- [W0 W1 W2]x5 then [-W0 -W1 -W2]x2 -- so the 21-slot
replication is done BY THE STORE DMA: per tile the +-W values are cast
to bf16 once ([W0|W1|W2] + a duplicate + [-W0|-W1|-W2]) and three DMAs
with stride-0 source dims fan them out to DRAM (contiguous 2880 B +
1152 B runs per row, 5 descriptors of >=576 B, so no small-write HBM
penalty). Compute engines only ever touch the 6 distinct fields.
"""

import numpy as np

RES = 96
N_CORES = 8
PLANES = RES // N_CORES        # 12
LROWS = PLANES * RES           # 1152 local rows
NT = LROWS // 128              # 9 local 128-row tiles
EXTNT = NT + 2                 # 11 extended tiles (halo)
NROWS_G = RES * RES            # 9216 global rows
NSL = 21                       # stored (nonzero) output slots
OW = NSL * RES                 # 2016 free elems per output row
F = RES // 2                   # 48: fp32 words per 96-bf16 slot run

# small bcast tile columns: kvec
B_KV = 0
BCW = RES

# 27-slot -> 21-slot compression, device order [P N P N P P P] where
# P = [+W0 +W1 +W2], N = [-W0 -W1 -W2]: [P|N] is contiguous in SBUF, so
# the store DMA covers slots 0..11 with two 1152 B descriptors per row.
# NZ_PERM[i] = original slot (s=9i'+3j+k) whose value the i-th stored slot is.
NZ_PERM = [0, 1, 2, 4, 9, 18, 10, 3, 6, 8, 17, 22,
           12, 13, 14, 20, 23, 16, 24, 25, 26]

HCHUNKS = [(0, 3), (3, 6), (6, 9), (9, 11)]   # ext-block ranges for phase A
# tiles emitted right after the phase-A chunk that completes their halo
CHUNK_TILES = [(0, 1), (1, 4), (4, 7), (7, 9)]


def _grid_x():
    # Match the reference grid bit-for-bit: jnp.linspace in fp32 on CPU.
    import jax
    import jax.numpy as jnp
    MAX_X = 1.0
    DX = np.float32(MAX_X / (RES / 2 - 1))

    def _ls():
        return jnp.linspace(
            DX * (1 - RES / 2), DX * (RES / 2 - 1), RES, dtype=jnp.float32
        )

    try:
        with jax.default_device(jax.devices("cpu")[0]):
            x = np.asarray(_ls())
    except Exception:
        x = np.asarray(_ls())
    return x, float(DX)


def _fd_sources(idx, coeff_c, coeff_e):
    """(offset, coeff) pairs for d/didx with 1st-order one-sided edges."""
    if idx == 0:
        return [(1, coeff_e), (0, -coeff_e)]
    if idx == RES - 1:
        return [(0, coeff_e), (-1, -coeff_e)]
    return [(1, coeff_c), (-1, -coeff_c)]


def _build_dmat(core, DX):
    """[128, 6*3*128] bf16 FD matrices as matmul lhsT ([q, p] = coeff of
    ext-row q in output row p); 0.5 Christoffel factor folded in. All
    values are +-0.25/DX or +-0.5/DX = +-11.75 / +-23.5, exact in bf16.
    Entries: 0 g0(t=0), 1 g0(interior), 2 g0(t=8), 3..5 g1(t%3)."""
    import ml_dtypes
    c0 = 0.5 * (1.0 / (2.0 * np.float64(DX)))
    ce = 0.5 * (1.0 / np.float64(DX))
    out = np.zeros((128, 6 * 3 * 128), np.float64)

    def fill(entry, t, axis):
        for p in range(128):
            gr = core * LROWS + 128 * t + p
            a = (gr // RES) if axis == 0 else (gr % RES)
            step = RES if axis == 0 else 1
            for off, cf in _fd_sources(a, c0, ce):
                g2 = gr + off * step
                e_ = g2 - core * LROWS + 128
                j = e_ // 128 - t
                q = e_ - 128 * (t + j)
                assert 0 <= j <= 2 and 0 <= q < 128, (core, t, p, off)
                out[q, (entry * 3 + j) * 128 + p] = cf

    fill(0, 0, 0)
    fill(1, 1, 0)
    fill(2, NT - 1, 0)
    for v in range(3):
        fill(3 + v, v, 1)
    return out.astype(ml_dtypes.bfloat16)


def _build_program():
    import dataclasses as _dc

    import concourse.bacc as bacc
    import concourse.mybir as mybir
    import concourse.tile as tile
    from concourse.alu_op_type import AluOpType

    DT = mybir.dt.float32
    BF = mybir.dt.bfloat16
    AF = mybir.ActivationFunctionType

    def shift(apv, off, dims):
        return _dc.replace(apv, offset=apv.offset + off, ap=[apv.ap[0]] + dims)

    nc = bacc.Bacc(None, target_bir_lowering=False, debug=True)
    R2W = 2 * RES + 2 * EXTNT * 128   # r2 rhs | r2 lhsT, one load
    d_r2 = nc.dram_tensor("r2in", [2, R2W], BF, kind="ExternalInput")
    d_b = nc.dram_tensor("bsm", [128, BCW], DT, kind="ExternalInput")
    d_dmat = nc.dram_tensor("dmat", [128, 6 * 3 * 128], BF, kind="ExternalInput")
    d_out = nc.dram_tensor("out", [LROWS, OW], BF, kind="ExternalOutput")

    HW_ = EXTNT * RES             # 1056: free width of the ext h field
    with tile.TileContext(nc) as tc:
        with (
            tc.tile_pool(name="const", bufs=1) as cpool,
            tc.tile_pool(name="chunk", bufs=3) as chpool,
            tc.tile_pool(name="work", bufs=4) as wpool,
            tc.tile_pool(name="w3b", bufs=6) as w3bpool,
            tc.tile_pool(name="psum", bufs=3, space="PSUM") as pspool,
            tc.tile_pool(name="psr2", bufs=2, space="PSUM") as r2pool,
        ):
            # --- constants in (tiny r2 operands first: phase A head starts
            # on them; the big FD-matrix load is only needed ~6us later and
            # goes on the scalar HWDGE queue so it doesn't delay the rest) ---
            r2t = cpool.tile([2, R2W], BF)
            nc.sync.dma_start(r2t[:], d_r2[:])
            B = cpool.tile([128, BCW], DT)
            nc.sync.dma_start(B[:], d_b[:])
            dm = cpool.tile([128, 6 * 3 * 128], BF)
            nc.sync.dma_start(dm[:], d_dmat[:])

            # consolidate the ACT table loads: touch the table-backed funcs
            # once, first thing, on a tiny tile
            dmy = cpool.tile([1, 2], DT)
            nc.vector.memset(dmy[:], 1.0)
            dmy2 = cpool.tile([1, 2], DT)
            nc.scalar.activation(dmy2[:], dmy[:], AF.Sqrt)
            nc.scalar.activation(dmy2[:], dmy[:], AF.Square)

            H = cpool.tile([128, HW_], DT)
            Hb = cpool.tile([128, HW_], BF)

            def emit_chunk(b0, b1):
                # phase A: h field on ext blocks [b0, b1)
                nb = b1 - b0
                W = nb * RES
                csl = slice(RES * b0, RES * b1)
                # r^2/c_n = ab/c + crow/c (scales folded on host): 2-row
                # matmuls; then ONE recip-pair + ONE unscaled sqrt gives
                # q_n = (m_n/2)/r_n for both BHs at once
                ri = chpool.tile([128, 2 * W], DT, tag="ri")
                for n in range(2):
                    ps = r2pool.tile([128, W], DT, tag=f"ps{n}")
                    for e in range(b0, b1):
                        lo = 2 * RES + (n * EXTNT + e) * 128
                        nc.tensor.matmul(
                            ps[:, RES * (e - b0):RES * (e - b0 + 1)],
                            r2t[:, lo:lo + 128], r2t[:, RES * n:RES * (n + 1)],
                            start=True, stop=True,
                        )
                    nc.vector.reciprocal_approx_fast(ri[:, n * W:(n + 1) * W], ps[:])
                q = chpool.tile([128, 2 * W], DT, tag="q")
                nc.scalar.activation(q[:], ri[:], AF.Sqrt)
                psi = chpool.tile([128, W], DT, tag="psi")
                nc.vector.scalar_tensor_tensor(
                    psi[:], q[:, 0:W], 1.0, q[:, W:2 * W], AluOpType.add, AluOpType.add
                )
                hsq = chpool.tile([128, W], DT, tag="hsq")
                nc.vector.tensor_mul(hsq[:], psi[:], psi[:])
                nc.vector.tensor_mul(H[:, csl], hsq[:], hsq[:])
                nc.scalar.activation(Hb[:, csl], hsq[:], AF.Square)

            def emit_tile(t):
                # FD matmuls, W fields, bf16 cast, replicating store
                g0e = 0 if t == 0 else (2 if t == NT - 1 else 1)
                g1e = 3 + (t % 3)
                hsl = slice(RES * (t + 1), RES * (t + 2))
                P = pspool.tile([128, 2 * RES], DT, tag="pp")
                for half, ge in ((0, g0e), (1, g1e)):
                    for j in range(3):
                        lhs = dm[:, (ge * 3 + j) * 128:(ge * 3 + j + 1) * 128]
                        rsl = slice(RES * (t + j), RES * (t + j + 1))
                        nc.tensor.matmul(
                            P[:, RES * half:RES * (half + 1)], lhs, Hb[:, rsl],
                            start=(j == 0), stop=(j == 2)
                        )

                hinv = wpool.tile([128, RES], DT, tag="hinv")
                nc.vector.reciprocal_approx_fast(hinv[:], H[:, hsl])
                hz = wpool.tile([128, RES], DT, tag="hz")
                nc.vector.tensor_mul(hz[:], hinv[:], B[:, B_KV:B_KV + RES])
                st = wpool.tile([128, RES], DT, tag="st")
                Ht = H[:, hsl]
                nc.gpsimd.tensor_sub(st[:, 1:95], Ht[:, 2:96], Ht[:, 0:94])
                # both one-sided edge columns in one op: st[{0,95}]
                e_d = _dc.replace(st[:], ap=[st[:].ap[0], [95, 2], [1, 1]])
                e_a = shift(Ht, 1, [[94, 2], [1, 1]])
                e_b = shift(Ht, 0, [[94, 2], [1, 1]])
                nc.gpsimd.tensor_sub(e_d, e_a, e_b)

                # W0|W1 in one op (hinv broadcast over the two PSUM halves)
                w3 = wpool.tile([128, 3 * RES], DT, tag="w3")
                hib = _dc.replace(
                    hinv[:], ap=[hinv[:].ap[0], [0, 2], [1, RES]]
                )
                w01 = w3[:, 0:2 * RES].rearrange("p (h z) -> p h z", z=RES)
                Pv = P[:].rearrange("p (h z) -> p h z", z=RES)
                nc.vector.tensor_mul(w01[:, :, :], Pv[:, :, :], hib)
                nc.vector.tensor_mul(w3[:, 2 * RES:3 * RES], st[:], hz[:])

                # bf16 once: w3b = [pos | neg], pos = [W0|W1|W2]; the cast is
                # ACT-engine, the negation a sign-bit XOR on the fp32 view
                w3b = w3bpool.tile([128, 6 * RES], BF, tag="w3b")
                nc.scalar.copy(w3b[:, 0:3 * RES], w3[:])
                posv = w3b[:, 0:3 * RES].bitcast(mybir.dt.uint32)
                negv = w3b[:, 3 * RES:6 * RES].bitcast(mybir.dt.uint32)
                nc.vector.tensor_scalar(
                    negv, posv, 0x80008000, None, AluOpType.bitwise_xor
                )

                # store, layout [P N P N P P P]: slots 12-20 as pos x3
                # (576 B descs, no neg dependency, issued from the scalar
                # HWDGE queue right after the cast), slots 0-11 as
                # [pos|neg]x2 (1152 B descs, sync queue) -- two issue
                # queues so HWDGE descriptor emission is not the pacer
                pos = w3b[:, 0:3 * RES]
                nc.scalar.dma_start(
                    d_out[128 * t:128 * (t + 1), 12 * RES:21 * RES],
                    _dc.replace(pos, ap=[pos.ap[0], [0, 3], [1, 3 * RES]]),
                    single_packet=True,
                )
                pn = w3b[:, 0:6 * RES]
                nc.sync.dma_start(
                    d_out[128 * t:128 * (t + 1), 0:12 * RES],
                    _dc.replace(pn, ap=[pn.ap[0], [0, 2], [1, 6 * RES]]),
                    single_packet=True,
                )

            # interleave: emit each tile right after its halo chunk so the
            # scalar queue serves tile casts before later chunks' sqrts
            for ci, (b0, b1) in enumerate(HCHUNKS):
                emit_chunk(b0, b1)
                for t in range(*CHUNK_TILES[ci]):
                    emit_tile(t)

    nc.finalize()
    return nc


def _build_static():
    x, DX = _grid_x()
    dmats = [_build_dmat(c, DX) for c in range(N_CORES)]
    kvec = np.full(RES, 0.25 / DX, np.float64)
    kvec[0] = kvec[-1] = 0.5 / DX
    return x, DX, dmats, kvec.astype(np.float32)


_CACHE = {}


def _get_setup():
    if "nc" not in _CACHE:
        _CACHE["static"] = _build_static()
        _CACHE["nc"] = _build_program()
    return _CACHE["nc"], _CACHE["static"]


def _build_inmaps(BH_positions, BH_masses_presoftplus, static):
    import ml_dtypes
    x, DX, dmats, kvec = static
    pos = np.asarray(BH_positions, np.float64).reshape(2, 3)
    pre = np.asarray(BH_masses_presoftplus, np.float32)
    masses = np.log1p(np.exp(pre)).astype(np.float64)
    xd = x.astype(np.float64)

    # small bcast tile (identical across cores): kvec
    bc = np.zeros((1, BCW), np.float32)
    bc[0, B_KV:B_KV + RES] = kvec
    bsm = np.ascontiguousarray(np.broadcast_to(bc, (128, BCW)))

    in_maps = []
    for c in range(N_CORES):
        slab = c * LROWS
        e = np.arange(EXTNT * 128)
        g = np.clip(slab - 128 + e, 0, NROWS_G - 1)
        xr = xd[g % RES]    # X coordinate (a1)
        yr = xd[g // RES]   # Y coordinate (a0)
        # one r2 tensor, mass scales c_n = (m_n/2)^2 folded in so the
        # matmul yields r^2/c_n directly (recip+sqrt then needs no scale):
        #   [rhs: per BH row0 = 1, row1 = (z-pz)^2]
        #   [lhsT: per (BH, ext block) row0 = ab/c, row1 = 1/c]
        r2 = np.zeros((2, 2 * RES + 2 * EXTNT * 128), np.float64)
        for n in range(2):
            cn = (masses[n] / 2.0) ** 2
            r2[0, RES * n:RES * (n + 1)] = 1.0
            r2[1, RES * n:RES * (n + 1)] = (xd - pos[n, 2]) ** 2
            ab = (xr - pos[n, 0]) ** 2 + (yr - pos[n, 1]) ** 2
            lo = 2 * RES + n * EXTNT * 128
            r2[0, lo:lo + EXTNT * 128] = ab / cn
            r2[1, lo:lo + EXTNT * 128] = 1.0 / cn
        in_maps.append({
            "r2in": r2.astype(ml_dtypes.bfloat16),
            "bsm": bsm,
            "dmat": dmats[c],
        })
    return in_maps


def kernel(BH_positions, BH_masses_presoftplus):
    from concourse.bass_utils import run_bass_kernel_spmd

    nc, static = _get_setup()
    in_maps = _build_inmaps(BH_positions, BH_masses_presoftplus, static)
    res = run_bass_kernel_spmd(nc, in_maps, list(range(N_CORES)))

    # host gather: insert zero slots, upcast bf16 -> f32, z-major reorder
    full = np.zeros((N_CORES * LROWS, 27, RES), np.float32)
    for c in range(N_CORES):
        part = np.asarray(res.results[c]["out"]).reshape(LROWS, NSL, RES)
        full[c * LROWS:(c + 1) * LROWS, NZ_PERM, :] = part
    out = full.reshape(RES, RES, 27, RES).transpose(0, 1, 3, 2)
    return np.ascontiguousarray(out).reshape(RES, RES, RES, 3, 3, 3)


# revision 44
# speedup vs baseline: 1.0108x; 1.0108x over previous
"""Trainium2 Bass kernel for the Brill-Lindquist Christoffel-symbol grid.

Math: the reference reduces to
    psi  = 1 + sum_n m_n / (2 r_n),   m = softplus(pre)
    h    = psi^4
    G_c  = finite-difference gradient of h along grid axis c (2nd order
           central interior, 1st order one-sided edges, spacing DX)
    W_c  = 0.5 * G_c / h
    Gamma^i_{jk} = delta_ij W_k + delta_ik W_j - delta_jk W_i
so the [96,96,96,3,3,3] output is +-W_c scattered over 27 slots per
point (21 nonzero, 6 identically zero).

Sharding: axis 0 (12 planes per core x 8 cores). h is analytic in the
inputs, so each core evaluates its slab plus a 1-plane halo directly --
no inter-core exchange. Per core the grid is row-packed: row = a0*96+a1
(1152 rows -> 9 tiles of 128 partitions), free dim = a2 (96); h lives on
an 11-tile extended row window (halo tiles at both ends).

r^2 = (x-px)^2+(y-py)^2 + (z-pz)^2 is an outer sum of a per-row and a
per-z term, so it is produced by tiny 2-row matmuls on the otherwise
idle tensor engine (lhsT = [ab_n | 1], rhs = [1 | crow_n]). The h field
is built once per core in fp32 (axis-2 shift FD, 1/h) and bf16 (matmul
operand). Axis-0/1 derivatives are bf16 matmuls against per-core FD
matrices (+-0.25/DX, +-0.5/DX exact in bf16; one-sided grid edges
folded in). bf16 rounding of h bounds the W error by ~2^-10/DX ~ 0.05
absolute vs the ~500 the 2e-2 gate allows.

Output: device stores only the 21 nonzero slots, slot-major bf16
([row, s*96+z], 4032 B/row); the host inserts the 6 zero slots, casts
to f32 and permutes to [...,z,3,3,3]. The compressed slot order is
value-major -- [W0 W1 W2]x5 then [-W0 -W1 -W2]x2 -- so the 21-slot
replication is done BY THE STORE DMA: per tile the +-W values are cast
to bf16 once ([W0|W1|W2] + a duplicate + [-W0|-W1|-W2]) and three DMAs
with stride-0 source dims fan them out to DRAM (contiguous 2880 B +
1152 B runs per row, 5 descriptors of >=576 B, so no small-write HBM
penalty). Compute engines only ever touch the 6 distinct fields.
"""

import numpy as np

RES = 96
N_CORES = 8
PLANES = RES // N_CORES        # 12
LROWS = PLANES * RES           # 1152 local rows
NT = LROWS // 128              # 9 local 128-row tiles
EXTNT = NT + 2                 # 11 extended tiles (halo)
NROWS_G = RES * RES            # 9216 global rows
NSL = 21                       # stored (nonzero) output slots
OW = NSL * RES                 # 2016 free elems per output row
F = RES // 2                   # 48: fp32 words per 96-bf16 slot run

# small bcast tile columns: kvec
B_KV = 0
BCW = RES
_DX64 = float(np.float32(1.0 / (RES / 2 - 1)))   # grid spacing, fp32-exact
KV_C = 0.25 / _DX64                              # central z-FD scale (x0.5)
KV_E = 0.5 / _DX64                               # one-sided edge scale

# 27-slot -> 21-slot compression, device order [P N P N P P P] where
# P = [+W0 +W1 +W2], N = [-W0 -W1 -W2]: [P|N] is contiguous in SBUF, so
# the store DMA covers slots 0..11 with two 1152 B descriptors per row.
# NZ_PERM[i] = original slot (s=9i'+3j+k) whose value the i-th stored slot is.
NZ_PERM = [0, 1, 2, 4, 9, 18, 10, 3, 6, 8, 17, 22,
           12, 13, 14, 20, 23, 16, 24, 25, 26]

HCHUNKS = [(0, 3), (3, 6), (6, 9), (9, 11)]   # ext-block ranges for phase A
# tiles emitted right after the phase-A chunk that completes their halo
CHUNK_TILES = [(0, 1), (1, 4), (4, 7), (7, 9)]


def _grid_x():
    # Match the reference grid bit-for-bit: jnp.linspace in fp32 on CPU.
    import jax
    import jax.numpy as jnp
    MAX_X = 1.0
    DX = np.float32(MAX_X / (RES / 2 - 1))

    def _ls():
        return jnp.linspace(
            DX * (1 - RES / 2), DX * (RES / 2 - 1), RES, dtype=jnp.float32
        )

    try:
        with jax.default_device(jax.devices("cpu")[0]):
            x = np.asarray(_ls())
    except Exception:
        x = np.asarray(_ls())
    return x, float(DX)


def _fd_sources(idx, coeff_c, coeff_e):
    """(offset, coeff) pairs for d/didx with 1st-order one-sided edges."""
    if idx == 0:
        return [(1, coeff_e), (0, -coeff_e)]
    if idx == RES - 1:
        return [(0, coeff_e), (-1, -coeff_e)]
    return [(1, coeff_c), (-1, -coeff_c)]


def _build_dmat(core, DX):
    """[128, 6*3*128] bf16 FD matrices as matmul lhsT ([q, p] = coeff of
    ext-row q in output row p); 0.5 Christoffel factor folded in. All
    values are +-0.25/DX or +-0.5/DX = +-11.75 / +-23.5, exact in bf16.
    Entries: 0 g0(t=0), 1 g0(interior), 2 g0(t=8), 3..5 g1(t%3)."""
    import ml_dtypes
    c0 = 0.5 * (1.0 / (2.0 * np.float64(DX)))
    ce = 0.5 * (1.0 / np.float64(DX))
    out = np.zeros((128, 6 * 3 * 128), np.float64)

    def fill(entry, t, axis):
        for p in range(128):
            gr = core * LROWS + 128 * t + p
            a = (gr // RES) if axis == 0 else (gr % RES)
            step = RES if axis == 0 else 1
            for off, cf in _fd_sources(a, c0, ce):
                g2 = gr + off * step
                e_ = g2 - core * LROWS + 128
                j = e_ // 128 - t
                q = e_ - 128 * (t + j)
                assert 0 <= j <= 2 and 0 <= q < 128, (core, t, p, off)
                out[q, (entry * 3 + j) * 128 + p] = cf

    fill(0, 0, 0)
    fill(1, 1, 0)
    fill(2, NT - 1, 0)
    for v in range(3):
        fill(3 + v, v, 1)
    return out.astype(ml_dtypes.bfloat16)


def _build_program():
    import dataclasses as _dc

    import concourse.bacc as bacc
    import concourse.mybir as mybir
    import concourse.tile as tile
    from concourse.alu_op_type import AluOpType

    DT = mybir.dt.float32
    BF = mybir.dt.bfloat16
    AF = mybir.ActivationFunctionType

    def shift(apv, off, dims):
        return _dc.replace(apv, offset=apv.offset + off, ap=[apv.ap[0]] + dims)

    nc = bacc.Bacc(None, target_bir_lowering=False, debug=True)
    R2W = 2 * RES + 2 * EXTNT * 128   # r2 rhs | r2 lhsT, one load
    d_r2 = nc.dram_tensor("r2in", [2, R2W], BF, kind="ExternalInput")
    d_dmat = nc.dram_tensor("dmat", [128, 6 * 3 * 128], BF, kind="ExternalInput")
    d_out = nc.dram_tensor("out", [LROWS, OW], BF, kind="ExternalOutput")

    HW_ = EXTNT * RES             # 1056: free width of the ext h field
    with tile.TileContext(nc) as tc:
        with (
            tc.tile_pool(name="const", bufs=1) as cpool,
            tc.tile_pool(name="chunk", bufs=3) as chpool,
            tc.tile_pool(name="work", bufs=4) as wpool,
            tc.tile_pool(name="w3b", bufs=6) as w3bpool,
            tc.tile_pool(name="psum", bufs=3, space="PSUM") as pspool,
            tc.tile_pool(name="psr2", bufs=2, space="PSUM") as r2pool,
        ):
            # --- constants in (tiny r2 operands first: phase A head starts
            # on them; the big FD-matrix load is only needed ~6us later and
            # goes on the scalar HWDGE queue so it doesn't delay the rest) ---
            r2t = cpool.tile([2, R2W], BF)
            nc.sync.dma_start(r2t[:], d_r2[:])
            dm = cpool.tile([128, 6 * 3 * 128], BF)
            nc.sync.dma_start(dm[:], d_dmat[:])

            # kvec (z-FD column scale) built on idle gpsimd: 0.25/DX
            # interior, 0.5/DX at the one-sided edges
            B = cpool.tile([128, BCW], DT)
            nc.gpsimd.memset(B[:, B_KV:B_KV + RES], KV_C)
            nc.gpsimd.memset(B[:, B_KV:B_KV + 1], KV_E)
            nc.gpsimd.memset(B[:, B_KV + RES - 1:B_KV + RES], KV_E)

            # consolidate the ACT table loads: touch the table-backed funcs
            # once, first thing, on a tiny tile
            dmy = cpool.tile([1, 2], DT)
            nc.vector.memset(dmy[:], 1.0)
            dmy2 = cpool.tile([1, 2], DT)
            nc.scalar.activation(dmy2[:], dmy[:], AF.Sqrt)
            nc.scalar.activation(dmy2[:], dmy[:], AF.Square)

            H = cpool.tile([128, HW_], DT)
            Hb = cpool.tile([128, HW_], BF)

            def emit_chunk(b0, b1):
                # phase A: h field on ext blocks [b0, b1)
                nb = b1 - b0
                W = nb * RES
                csl = slice(RES * b0, RES * b1)
                # r^2/c_n = ab/c + crow/c (scales folded on host): 2-row
                # matmuls; then ONE recip-pair + ONE unscaled sqrt gives
                # q_n = (m_n/2)/r_n for both BHs at once
                ri = chpool.tile([128, 2 * W], DT, tag="ri")
                for n in range(2):
                    ps = r2pool.tile([128, W], DT, tag=f"ps{n}")
                    for e in range(b0, b1):
                        lo = 2 * RES + (n * EXTNT + e) * 128
                        nc.tensor.matmul(
                            ps[:, RES * (e - b0):RES * (e - b0 + 1)],
                            r2t[:, lo:lo + 128], r2t[:, RES * n:RES * (n + 1)],
                            start=True, stop=True,
                        )
                    nc.vector.reciprocal_approx_fast(ri[:, n * W:(n + 1) * W], ps[:])
                q = chpool.tile([128, 2 * W], DT, tag="q")
                nc.scalar.activation(q[:], ri[:], AF.Sqrt)
                psi = chpool.tile([128, W], DT, tag="psi")
                nc.vector.scalar_tensor_tensor(
                    psi[:], q[:, 0:W], 1.0, q[:, W:2 * W], AluOpType.add, AluOpType.add
                )
                hsq = chpool.tile([128, W], DT, tag="hsq")
                nc.gpsimd.tensor_mul(hsq[:], psi[:], psi[:])
                nc.gpsimd.tensor_mul(H[:, csl], hsq[:], hsq[:])
                nc.scalar.activation(Hb[:, csl], hsq[:], AF.Square)

            def emit_tile(t):
                # FD matmuls, W fields, bf16 cast, replicating store
                g0e = 0 if t == 0 else (2 if t == NT - 1 else 1)
                g1e = 3 + (t % 3)
                hsl = slice(RES * (t + 1), RES * (t + 2))
                P = pspool.tile([128, 2 * RES], DT, tag="pp")
                for half, ge in ((0, g0e), (1, g1e)):
                    for j in range(3):
                        lhs = dm[:, (ge * 3 + j) * 128:(ge * 3 + j + 1) * 128]
                        rsl = slice(RES * (t + j), RES * (t + j + 1))
                        nc.tensor.matmul(
                            P[:, RES * half:RES * (half + 1)], lhs, Hb[:, rsl],
                            start=(j == 0), stop=(j == 2)
                        )

                hinv = wpool.tile([128, RES], DT, tag="hinv")
                nc.vector.reciprocal_approx_fast(hinv[:], H[:, hsl])
                hz = wpool.tile([128, RES], DT, tag="hz")
                nc.vector.tensor_mul(hz[:], hinv[:], B[:, B_KV:B_KV + RES])
                st = wpool.tile([128, RES], DT, tag="st")
                Ht = H[:, hsl]
                nc.gpsimd.tensor_sub(st[:, 1:95], Ht[:, 2:96], Ht[:, 0:94])
                # both one-sided edge columns in one op: st[{0,95}]
                e_d = _dc.replace(st[:], ap=[st[:].ap[0], [95, 2], [1, 1]])
                e_a = shift(Ht, 1, [[94, 2], [1, 1]])
                e_b = shift(Ht, 0, [[94, 2], [1, 1]])
                nc.gpsimd.tensor_sub(e_d, e_a, e_b)

                # W0|W1 in one op (hinv broadcast over the two PSUM halves)
                w3 = wpool.tile([128, 3 * RES], DT, tag="w3")
                hib = _dc.replace(
                    hinv[:], ap=[hinv[:].ap[0], [0, 2], [1, RES]]
                )
                w01 = w3[:, 0:2 * RES].rearrange("p (h z) -> p h z", z=RES)
                Pv = P[:].rearrange("p (h z) -> p h z", z=RES)
                nc.vector.tensor_mul(w01[:, :, :], Pv[:, :, :], hib)
                nc.vector.tensor_mul(w3[:, 2 * RES:3 * RES], st[:], hz[:])

                # bf16 once: w3b = [pos | neg], pos = [W0|W1|W2]; the cast is
                # ACT-engine, the negation a sign-bit XOR on the fp32 view
                w3b = w3bpool.tile([128, 6 * RES], BF, tag="w3b")
                nc.scalar.copy(w3b[:, 0:3 * RES], w3[:])
                posv = w3b[:, 0:3 * RES].bitcast(mybir.dt.uint32)
                negv = w3b[:, 3 * RES:6 * RES].bitcast(mybir.dt.uint32)
                nc.vector.tensor_scalar(
                    negv, posv, 0x80008000, None, AluOpType.bitwise_xor
                )

                # store, layout [P N P N P P P]: slots 12-20 as pos x3
                # (576 B descs, no neg dependency, issued from the scalar
                # HWDGE queue right after the cast), slots 0-11 as
                # [pos|neg]x2 (1152 B descs, sync queue) -- two issue
                # queues so HWDGE descriptor emission is not the pacer
                pos = w3b[:, 0:3 * RES]
                nc.scalar.dma_start(
                    d_out[128 * t:128 * (t + 1), 12 * RES:21 * RES],
                    _dc.replace(pos, ap=[pos.ap[0], [0, 3], [1, 3 * RES]]),
                    single_packet=True,
                )
                pn = w3b[:, 0:6 * RES]
                nc.sync.dma_start(
                    d_out[128 * t:128 * (t + 1), 0:12 * RES],
                    _dc.replace(pn, ap=[pn.ap[0], [0, 2], [1, 6 * RES]]),
                    single_packet=True,
                )

            # interleave: emit each tile right after its halo chunk so the
            # scalar queue serves tile casts before later chunks' sqrts
            for ci, (b0, b1) in enumerate(HCHUNKS):
                emit_chunk(b0, b1)
                for t in range(*CHUNK_TILES[ci]):
                    emit_tile(t)

    nc.finalize()
    return nc


def _build_static():
    x, DX = _grid_x()
    dmats = [_build_dmat(c, DX) for c in range(N_CORES)]
    kvec = np.full(RES, 0.25 / DX, np.float64)
    kvec[0] = kvec[-1] = 0.5 / DX
    return x, DX, dmats, kvec.astype(np.float32)


_CACHE = {}


def _get_setup():
    if "nc" not in _CACHE:
        _CACHE["static"] = _build_static()
        _CACHE["nc"] = _build_program()
    return _CACHE["nc"], _CACHE["static"]


def _build_inmaps(BH_positions, BH_masses_presoftplus, static):
    import ml_dtypes
    x, DX, dmats, kvec = static
    pos = np.asarray(BH_positions, np.float64).reshape(2, 3)
    pre = np.asarray(BH_masses_presoftplus, np.float32)
    masses = np.log1p(np.exp(pre)).astype(np.float64)
    xd = x.astype(np.float64)

    in_maps = []
    for c in range(N_CORES):
        slab = c * LROWS
        e = np.arange(EXTNT * 128)
        g = np.clip(slab - 128 + e, 0, NROWS_G - 1)
        xr = xd[g % RES]    # X coordinate (a1)
        yr = xd[g // RES]   # Y coordinate (a0)
        # one r2 tensor, mass scales c_n = (m_n/2)^2 folded in so the
        # matmul yields r^2/c_n directly (recip+sqrt then needs no scale):
        #   [rhs: per BH row0 = 1, row1 = (z-pz)^2]
        #   [lhsT: per (BH, ext block) row0 = ab/c, row1 = 1/c]
        r2 = np.zeros((2, 2 * RES + 2 * EXTNT * 128), np.float64)
        for n in range(2):
            cn = (masses[n] / 2.0) ** 2
            r2[0, RES * n:RES * (n + 1)] = 1.0
            r2[1, RES * n:RES * (n + 1)] = (xd - pos[n, 2]) ** 2
            ab = (xr - pos[n, 0]) ** 2 + (yr - pos[n, 1]) ** 2
            lo = 2 * RES + n * EXTNT * 128
            r2[0, lo:lo + EXTNT * 128] = ab / cn
            r2[1, lo:lo + EXTNT * 128] = 1.0 / cn
        in_maps.append({
            "r2in": r2.astype(ml_dtypes.bfloat16),
            "dmat": dmats[c],
        })
    return in_maps


def kernel(BH_positions, BH_masses_presoftplus):
    from concourse.bass_utils import run_bass_kernel_spmd

    nc, static = _get_setup()
    in_maps = _build_inmaps(BH_positions, BH_masses_presoftplus, static)
    res = run_bass_kernel_spmd(nc, in_maps, list(range(N_CORES)))

    # host gather: insert zero slots, upcast bf16 -> f32, z-major reorder
    full = np.zeros((N_CORES * LROWS, 27, RES), np.float32)
    for c in range(N_CORES):
        part = np.asarray(res.results[c]["out"]).reshape(LROWS, NSL, RES)
        full[c * LROWS:(c + 1) * LROWS, NZ_PERM, :] = part
    out = full.reshape(RES, RES, 27, RES).transpose(0, 1, 3, 2)
    return np.ascontiguousarray(out).reshape(RES, RES, RES, 3, 3, 3)


# revision 46
# speedup vs baseline: 1.1188x; 1.1068x over previous
"""Trainium2 Bass kernel for the Brill-Lindquist Christoffel-symbol grid.

Math: the reference reduces to
    psi  = 1 + sum_n m_n / (2 r_n),   m = softplus(pre)
    h    = psi^4
    G_c  = finite-difference gradient of h along grid axis c (2nd order
           central interior, 1st order one-sided edges, spacing DX)
    W_c  = 0.5 * G_c / h
    Gamma^i_{jk} = delta_ij W_k + delta_ik W_j - delta_jk W_i
so the [96,96,96,3,3,3] output is +-W_c scattered over 27 slots per
point (21 nonzero, 6 identically zero).

Sharding: axis 0 (12 planes per core x 8 cores). h is analytic in the
inputs, so each core evaluates its slab plus a 1-plane halo directly --
no inter-core exchange. Per core the grid is row-packed: row = a0*96+a1
(1152 rows -> 9 tiles of 128 partitions), free dim = a2 (96); h lives on
an 11-tile extended row window (halo tiles at both ends).

r^2 = (x-px)^2+(y-py)^2 + (z-pz)^2 is an outer sum of a per-row and a
per-z term, so it is produced by tiny 2-row matmuls on the otherwise
idle tensor engine (lhsT = [ab_n | 1], rhs = [1 | crow_n]). The h field
is built once per core in fp32 (axis-2 shift FD, 1/h) and bf16 (matmul
operand). Axis-0/1 derivatives are bf16 matmuls against per-core FD
matrices (+-0.25/DX, +-0.5/DX exact in bf16; one-sided grid edges
folded in). bf16 rounding of h bounds the W error by ~2^-10/DX ~ 0.05
absolute vs the ~500 the 2e-2 gate allows.

Output: device stores only the 21 nonzero slots, slot-major bf16
([row, s*96+z], 4032 B/row); the host inserts the 6 zero slots, casts
to f32 and permutes to [...,z,3,3,3]. The compressed slot order is
[P N P N P P P] with P = [+W0|+W1|+W2], N = [-W0|-W1|-W2], so the
21-slot replication is done BY THE STORE DMA: per tile the W values are
cast to bf16 once (P via an ACT-engine copy, N via a sign-bit XOR on
the uint32 view) and two DMAs with stride-0 source dims fan them out to
DRAM -- [P|N]x2 as 1152 B descriptors from one HWDGE queue, Px3 as
576 B descriptors from the other, so neither descriptor emission nor
small HBM writes become the pacer. Compute engines only ever touch the
6 distinct fields; tiles are emitted interleaved with the phase-A
chunks that complete their halo so the store stream starts early.
"""

import numpy as np

RES = 96
N_CORES = 8
PLANES = RES // N_CORES        # 12
LROWS = PLANES * RES           # 1152 local rows
NT = LROWS // 128              # 9 local 128-row tiles
EXTNT = NT + 2                 # 11 extended tiles (halo)
NROWS_G = RES * RES            # 9216 global rows
NSL = 21                       # stored (nonzero) output slots
OW = NSL * RES                 # 2016 free elems per output row
F = RES // 2                   # 48: fp32 words per 96-bf16 slot run

# small bcast tile columns: kvec
B_KV = 0
BCW = RES
_DX64 = float(np.float32(1.0 / (RES / 2 - 1)))   # grid spacing, fp32-exact
KV_C = 0.25 / _DX64                              # central z-FD scale (x0.5)
KV_E = 0.5 / _DX64                               # one-sided edge scale

# 27-slot -> 21-slot compression, device order [P N P N P P P] where
# P = [+W0 +W1 +W2], N = [-W0 -W1 -W2]: [P|N] is contiguous in SBUF, so
# the store DMA covers slots 0..11 with two 1152 B descriptors per row.
# NZ_PERM[i] = original slot (s=9i'+3j+k) whose value the i-th stored slot is.
NZ_PERM = [0, 1, 2, 4, 9, 18, 10, 3, 6, 8, 17, 22,
           12, 13, 14, 20, 23, 16, 24, 25, 26]

HCHUNKS = [(0, 3), (3, 6), (6, 9), (9, 11)]   # ext-block ranges for phase A
# tiles emitted right after the phase-A chunk that completes their halo
CHUNK_TILES = [(0, 1), (1, 4), (4, 7), (7, 9)]


def _grid_x():
    # Match the reference grid bit-for-bit: jnp.linspace in fp32 on CPU.
    import jax
    import jax.numpy as jnp
    MAX_X = 1.0
    DX = np.float32(MAX_X / (RES / 2 - 1))

    def _ls():
        return jnp.linspace(
            DX * (1 - RES / 2), DX * (RES / 2 - 1), RES, dtype=jnp.float32
        )

    try:
        with jax.default_device(jax.devices("cpu")[0]):
            x = np.asarray(_ls())
    except Exception:
        x = np.asarray(_ls())
    return x, float(DX)


def _fd_sources(idx, coeff_c, coeff_e):
    """(offset, coeff) pairs for d/didx with 1st-order one-sided edges."""
    if idx == 0:
        return [(1, coeff_e), (0, -coeff_e)]
    if idx == RES - 1:
        return [(0, coeff_e), (-1, -coeff_e)]
    return [(1, coeff_c), (-1, -coeff_c)]


def _build_dmat(core, DX):
    """[128, 6*3*128] bf16 FD matrices as matmul lhsT ([q, p] = coeff of
    ext-row q in output row p); 0.5 Christoffel factor folded in. All
    values are +-0.25/DX or +-0.5/DX = +-11.75 / +-23.5, exact in bf16.
    Entries: 0 g0(t=0), 1 g0(interior), 2 g0(t=8), 3..5 g1(t%3)."""
    import ml_dtypes
    c0 = 0.5 * (1.0 / (2.0 * np.float64(DX)))
    ce = 0.5 * (1.0 / np.float64(DX))
    out = np.zeros((128, 6 * 3 * 128), np.float64)

    def fill(entry, t, axis):
        for p in range(128):
            gr = core * LROWS + 128 * t + p
            a = (gr // RES) if axis == 0 else (gr % RES)
            step = RES if axis == 0 else 1
            for off, cf in _fd_sources(a, c0, ce):
                g2 = gr + off * step
                e_ = g2 - core * LROWS + 128
                j = e_ // 128 - t
                q = e_ - 128 * (t + j)
                assert 0 <= j <= 2 and 0 <= q < 128, (core, t, p, off)
                out[q, (entry * 3 + j) * 128 + p] = cf

    fill(0, 0, 0)
    fill(1, 1, 0)
    fill(2, NT - 1, 0)
    for v in range(3):
        fill(3 + v, v, 1)
    return out.astype(ml_dtypes.bfloat16)


def _build_program():
    import dataclasses as _dc

    import concourse.bacc as bacc
    import concourse.mybir as mybir
    import concourse.tile as tile
    from concourse.alu_op_type import AluOpType

    DT = mybir.dt.float32
    BF = mybir.dt.bfloat16
    AF = mybir.ActivationFunctionType

    def shift(apv, off, dims):
        return _dc.replace(apv, offset=apv.offset + off, ap=[apv.ap[0]] + dims)

    nc = bacc.Bacc(None, target_bir_lowering=False, debug=True)
    R2W = 2 * RES + 2 * EXTNT * 128   # r2 rhs | r2 lhsT, one load
    d_r2 = nc.dram_tensor("r2in", [2, R2W], BF, kind="ExternalInput")
    d_dmat = nc.dram_tensor("dmat", [128, 6 * 3 * 128], BF, kind="ExternalInput")
    d_out = nc.dram_tensor("out", [LROWS, OW], BF, kind="ExternalOutput")

    HW_ = EXTNT * RES             # 1056: free width of the ext h field
    with tile.TileContext(nc) as tc:
        with (
            tc.tile_pool(name="const", bufs=1) as cpool,
            tc.tile_pool(name="chunk", bufs=4) as chpool,
            tc.tile_pool(name="work", bufs=6) as wpool,
            tc.tile_pool(name="w3b", bufs=9) as w3bpool,
            tc.tile_pool(name="psum", bufs=3, space="PSUM") as pspool,
            tc.tile_pool(name="psr2", bufs=2, space="PSUM") as r2pool,
        ):
            # --- constants in (tiny r2 operands first: phase A head starts
            # on them; the big FD-matrix load is only needed ~6us later and
            # goes on the scalar HWDGE queue so it doesn't delay the rest) ---
            r2t = cpool.tile([2, R2W], BF)
            nc.sync.dma_start(r2t[:], d_r2[:])
            dm = cpool.tile([128, 6 * 3 * 128], BF)
            nc.sync.dma_start(dm[:], d_dmat[:])

            # kvec (z-FD column scale) built on idle gpsimd: 0.25/DX
            # interior, 0.5/DX at the one-sided edges
            B = cpool.tile([128, BCW], DT)
            nc.gpsimd.memset(B[:, B_KV:B_KV + RES], KV_C)
            nc.gpsimd.memset(B[:, B_KV:B_KV + 1], KV_E)
            nc.gpsimd.memset(B[:, B_KV + RES - 1:B_KV + RES], KV_E)

            # consolidate the ACT table loads: touch the table-backed funcs
            # once, first thing, on a tiny tile
            dmy = cpool.tile([1, 2], DT)
            nc.vector.memset(dmy[:], 1.0)
            dmy2 = cpool.tile([1, 2], DT)
            nc.scalar.activation(dmy2[:], dmy[:], AF.Sqrt)
            nc.scalar.activation(dmy2[:], dmy[:], AF.Square)

            H = cpool.tile([128, HW_], DT)
            Hb = cpool.tile([128, HW_], BF)

            def emit_chunk(b0, b1):
                # phase A: h field on ext blocks [b0, b1)
                nb = b1 - b0
                W = nb * RES
                csl = slice(RES * b0, RES * b1)
                # r^2/c_n = ab/c + crow/c (scales folded on host): 2-row
                # matmuls; then ONE recip-pair + ONE unscaled sqrt gives
                # q_n = (m_n/2)/r_n for both BHs at once
                ri = chpool.tile([128, 2 * W], DT, tag="ri")
                for n in range(2):
                    ps = r2pool.tile([128, W], DT, tag=f"ps{n}")
                    for e in range(b0, b1):
                        lo = 2 * RES + (n * EXTNT + e) * 128
                        nc.tensor.matmul(
                            ps[:, RES * (e - b0):RES * (e - b0 + 1)],
                            r2t[:, lo:lo + 128], r2t[:, RES * n:RES * (n + 1)],
                            start=True, stop=True,
                        )
                    nc.vector.reciprocal_approx_fast(ri[:, n * W:(n + 1) * W], ps[:])
                q = chpool.tile([128, 2 * W], DT, tag="q")
                nc.scalar.activation(q[:], ri[:], AF.Sqrt)
                psi = chpool.tile([128, W], DT, tag="psi")
                nc.vector.scalar_tensor_tensor(
                    psi[:], q[:, 0:W], 1.0, q[:, W:2 * W], AluOpType.add, AluOpType.add
                )
                hsq = chpool.tile([128, W], DT, tag="hsq")
                nc.gpsimd.tensor_mul(hsq[:], psi[:], psi[:])
                nc.gpsimd.tensor_mul(H[:, csl], hsq[:], hsq[:])
                nc.scalar.activation(Hb[:, csl], hsq[:], AF.Square)

            def emit_tile(t):
                # FD matmuls, W fields, bf16 cast, replicating store
                g0e = 0 if t == 0 else (2 if t == NT - 1 else 1)
                g1e = 3 + (t % 3)
                hsl = slice(RES * (t + 1), RES * (t + 2))
                P = pspool.tile([128, 2 * RES], DT, tag="pp")
                for half, ge in ((0, g0e), (1, g1e)):
                    for j in range(3):
                        lhs = dm[:, (ge * 3 + j) * 128:(ge * 3 + j + 1) * 128]
                        rsl = slice(RES * (t + j), RES * (t + j + 1))
                        nc.tensor.matmul(
                            P[:, RES * half:RES * (half + 1)], lhs, Hb[:, rsl],
                            start=(j == 0), stop=(j == 2)
                        )

                hinv = wpool.tile([128, RES], DT, tag="hinv")
                nc.vector.reciprocal_approx_fast(hinv[:], H[:, hsl])
                hz = wpool.tile([128, RES], DT, tag="hz")
                nc.vector.tensor_mul(hz[:], hinv[:], B[:, B_KV:B_KV + RES])
                st = wpool.tile([128, RES], DT, tag="st")
                Ht = H[:, hsl]
                nc.gpsimd.tensor_sub(st[:, 1:95], Ht[:, 2:96], Ht[:, 0:94])
                # both one-sided edge columns in one op: st[{0,95}]
                e_d = _dc.replace(st[:], ap=[st[:].ap[0], [95, 2], [1, 1]])
                e_a = shift(Ht, 1, [[94, 2], [1, 1]])
                e_b = shift(Ht, 0, [[94, 2], [1, 1]])
                nc.gpsimd.tensor_sub(e_d, e_a, e_b)

                # W0|W1 in one op (hinv broadcast over the two PSUM halves)
                w3 = wpool.tile([128, 3 * RES], DT, tag="w3")
                hib = _dc.replace(
                    hinv[:], ap=[hinv[:].ap[0], [0, 2], [1, RES]]
                )
                w01 = w3[:, 0:2 * RES].rearrange("p (h z) -> p h z", z=RES)
                Pv = P[:].rearrange("p (h z) -> p h z", z=RES)
                nc.vector.tensor_mul(w01[:, :, :], Pv[:, :, :], hib)
                nc.vector.tensor_mul(w3[:, 2 * RES:3 * RES], st[:], hz[:])

                # bf16 once: w3b = [pos | neg], pos = [W0|W1|W2]; the cast is
                # ACT-engine, the negation a sign-bit XOR on the fp32 view
                w3b = w3bpool.tile([128, 6 * RES], BF, tag="w3b")
                nc.scalar.copy(w3b[:, 0:3 * RES], w3[:])
                posv = w3b[:, 0:3 * RES].bitcast(mybir.dt.uint32)
                negv = w3b[:, 3 * RES:6 * RES].bitcast(mybir.dt.uint32)
                nc.vector.tensor_scalar(
                    negv, posv, 0x80008000, None, AluOpType.bitwise_xor
                )

                # store, layout [P N P N P P P]: slots 12-20 as pos x3
                # (576 B descs, no neg dependency, issued from the scalar
                # HWDGE queue right after the cast), slots 0-11 as
                # [pos|neg]x2 (1152 B descs, sync queue) -- two issue
                # queues so HWDGE descriptor emission is not the pacer
                pos = w3b[:, 0:3 * RES]
                nc.scalar.dma_start(
                    d_out[128 * t:128 * (t + 1), 12 * RES:21 * RES],
                    _dc.replace(pos, ap=[pos.ap[0], [0, 3], [1, 3 * RES]]),
                    single_packet=True,
                )
                pn = w3b[:, 0:6 * RES]
                nc.sync.dma_start(
                    d_out[128 * t:128 * (t + 1), 0:12 * RES],
                    _dc.replace(pn, ap=[pn.ap[0], [0, 2], [1, 6 * RES]]),
                    single_packet=True,
                )

            # interleave: emit each tile right after its halo chunk so the
            # scalar queue serves tile casts before later chunks' sqrts
            for ci, (b0, b1) in enumerate(HCHUNKS):
                emit_chunk(b0, b1)
                for t in range(*CHUNK_TILES[ci]):
                    emit_tile(t)

    nc.finalize()
    return nc


def _build_static():
    x, DX = _grid_x()
    dmats = [_build_dmat(c, DX) for c in range(N_CORES)]
    kvec = np.full(RES, 0.25 / DX, np.float64)
    kvec[0] = kvec[-1] = 0.5 / DX
    return x, DX, dmats, kvec.astype(np.float32)


_CACHE = {}


def _get_setup():
    if "nc" not in _CACHE:
        _CACHE["static"] = _build_static()
        _CACHE["nc"] = _build_program()
    return _CACHE["nc"], _CACHE["static"]


def _build_inmaps(BH_positions, BH_masses_presoftplus, static):
    import ml_dtypes
    x, DX, dmats, kvec = static
    pos = np.asarray(BH_positions, np.float64).reshape(2, 3)
    pre = np.asarray(BH_masses_presoftplus, np.float32)
    masses = np.log1p(np.exp(pre)).astype(np.float64)
    xd = x.astype(np.float64)

    in_maps = []
    for c in range(N_CORES):
        slab = c * LROWS
        e = np.arange(EXTNT * 128)
        g = np.clip(slab - 128 + e, 0, NROWS_G - 1)
        xr = xd[g % RES]    # X coordinate (a1)
        yr = xd[g // RES]   # Y coordinate (a0)
        # one r2 tensor, mass scales c_n = (m_n/2)^2 folded in so the
        # matmul yields r^2/c_n directly (recip+sqrt then needs no scale):
        #   [rhs: per BH row0 = 1, row1 = (z-pz)^2]
        #   [lhsT: per (BH, ext block) row0 = ab/c, row1 = 1/c]
        r2 = np.zeros((2, 2 * RES + 2 * EXTNT * 128), np.float64)
        for n in range(2):
            cn = (masses[n] / 2.0) ** 2
            r2[0, RES * n:RES * (n + 1)] = 1.0
            r2[1, RES * n:RES * (n + 1)] = (xd - pos[n, 2]) ** 2
            ab = (xr - pos[n, 0]) ** 2 + (yr - pos[n, 1]) ** 2
            lo = 2 * RES + n * EXTNT * 128
            r2[0, lo:lo + EXTNT * 128] = ab / cn
            r2[1, lo:lo + EXTNT * 128] = 1.0 / cn
        in_maps.append({
            "r2in": r2.astype(ml_dtypes.bfloat16),
            "dmat": dmats[c],
        })
    return in_maps


def kernel(BH_positions, BH_masses_presoftplus):
    from concourse.bass_utils import run_bass_kernel_spmd

    nc, static = _get_setup()
    in_maps = _build_inmaps(BH_positions, BH_masses_presoftplus, static)
    res = run_bass_kernel_spmd(nc, in_maps, list(range(N_CORES)))

    # host gather: insert zero slots, upcast bf16 -> f32, z-major reorder
    full = np.zeros((N_CORES * LROWS, 27, RES), np.float32)
    for c in range(N_CORES):
        part = np.asarray(res.results[c]["out"]).reshape(LROWS, NSL, RES)
        full[c * LROWS:(c + 1) * LROWS, NZ_PERM, :] = part
    out = full.reshape(RES, RES, 27, RES).transpose(0, 1, 3, 2)
    return np.ascontiguousarray(out).reshape(RES, RES, RES, 3, 3, 3)
